# revision 27
# baseline (speedup 1.0000x reference)
"""Trainium2 Bass kernel for the DifferentiableMemory scatter_memory problem.

Data-parallel over 8 NeuronCores: batch B=32768 is sharded into 8 x 4096 rows.
Host side does layout only (transpose/cast/concat/weight repack); all NN math
(encoder MLP, cosine sims, top-k, importance net) runs on device with fp32
PSUM accumulation.

Device dataflow (per core, 8 superblocks of 512 batch columns):
  activations live transposed [feature, batch]:
    xT        [128, 6, 512]  cue.T chunks in fp8 e4m3 (x32 host prescale),
                DRAM laid out per-superblock-contiguous
    h1T       = gelu((W1.T @ xT)/S + b1)    -> [256, 512] bf16, W1 fp8 x512,
                3 DoubleRow matmuls per output tile, weights packed pair-major
                so the first chunk-pair's 48KB lands early and matmuls start
                ~2us sooner than a monolithic weight load
    encT      = W2.T @ h1T + b2             -> [128, 512] bf16 (one fp8 DR
                matmul; descale+bias ride the ACT Identity)
    enc2      = encT^2 via ACT Square (same gelu_and_others table: no swaps);
                ssq[b] = ones.T @ enc2 on PE; rsqrt = quake seed + 1 Newton
                step (rel err ~2e-3, far under the output budget)
    sims[b,n] = encT.T @ centT_scaled -> [128, 500] fp32 PSUM (centT
                pre-divided by ||c||; divide by ||enc|| AFTER top-8: positive
                per-row scale preserves order), top8 = one DVE max8 per tile
    impT      = sigmoid(w2i.T @ gelu((W1i.T @ xT)/S + b1i) + b2i) * mean(emo)
                sigmoid as polynomial 0.5+z/4-z^3/48 (|z|<0.25, err <1e-5);
                the /4 of mean(emo) is folded into the polynomial

  Epilogue is pipelined off the critical path: rsqrt/sigmoid chains run in
  DVE slack (tiles 0:16 at sb5) or on the idle GpSimd (16:24 at sb6, 24:32
  at sb7), and the output ships in three staged DMAs (tiles 0:16 during sb6,
  16:28 during sb7's matmuls, 28:32 right after the last max8) so the
  tail after the final matmul is just max8 -> one mul -> one small DMA.
  A PE warm-up burst (gated by a GpSimd memset, ~1.1us earlier than a DVE
  one) rides out the HAM clock ramp before the first real matmuls.
"""

import numpy as np
import ml_dtypes

BF16 = ml_dtypes.bfloat16
FP8 = ml_dtypes.float8_e4m3

N_CORES = 8
B = 32768
BL = B // N_CORES          # 4096 rows per core
SB = 512                   # superblock: batch columns per iteration
NSB = BL // SB             # 8 superblocks
Q = SB // 128              # 4 x 128-row tiles per superblock
D = 768
H1 = 256
E = 128
N = 500
K = 5
TOT = 902
DCH = D // 128             # 6
NPAIR = DCH // 2           # 3 chunk-pairs for DoubleRow
MW = H1 + 64               # 320 fused layer-1 output cols (h0|h1|imp)
SC_X = 32.0                # fp8 prescale on cue
SC_W = 512.0               # fp8 prescale on layer-1 weights
SINV = 1.0 / (SC_X * SC_W)
NWARM = 8

_CACHE = {}


def _build_nc(has_ist):
    """Build the device kernel. has_ist: include the internal_state chunk
    (False when it is all-zeros, making its contribution exactly zero)."""
    import concourse.bacc as bacc
    import concourse.bass as bass
    import concourse.tile as tile
    from concourse import mybir

    f32 = mybir.dt.float32
    bf16 = mybir.dt.bfloat16
    f8 = mybir.dt.float8e4
    i32 = mybir.dt.int32
    AF = mybir.ActivationFunctionType
    AO = mybir.AluOpType
    DR = mybir.MatmulPerfMode.DoubleRow
    ts = bass.ts

    nc = bacc.Bacc(None, target_bir_lowering=False,
               enable_asserts=False, enable_partition_id=False)

    # cue pre-chunked on host: cueP[p, sb, c, b] = cue[sb*SB+b, c*128+p]
    cueP = nc.dram_tensor("cueP", [128, NSB, DCH, SB], f8, kind="ExternalInput")
    tailT = nc.dram_tensor("tailT", [6, BL], bf16, kind="ExternalInput")
    if has_ist:
        istT = nc.dram_tensor("istT", [E, BL], bf16, kind="ExternalInput")
    emo = nc.dram_tensor("emo", [128, BL // 128, 4], f32, kind="ExternalInput")
    # fused layer-1 weights, chunk-PAIR major so each DR pass's weights are
    # one small contiguous DMA: w1P[p, c, j, m] = w1[p, 2c+j, m]
    w1P = nc.dram_tensor("w1P", [128, NPAIR, 2, MW], f8, kind="ExternalInput")
    w2 = nc.dram_tensor("w2", [128, 2, E], f8, kind="ExternalInput")
    iw1 = nc.dram_tensor("iw1", [128, 2 if has_ist else 1, 64], bf16,
                         kind="ExternalInput")
    iw2 = nc.dram_tensor("iw2", [64, 1], bf16, kind="ExternalInput")
    # bias_all: col 0-1 = enc_b1 halves, col 2 = enc_b2, col 3 = imp_b2
    bias = nc.dram_tensor("bias", [128, 4], f32, kind="ExternalInput")
    ib1 = nc.dram_tensor("ib1", [64, 1], f32, kind="ExternalInput")
    centT = nc.dram_tensor("centT", [128, N], bf16, kind="ExternalInput")
    out = nc.dram_tensor("out", [128, (BL // 128) * (K + 1)], f32,
                         kind="ExternalOutput")

    XT = NSB * Q  # 32 tiles of 128 rows

    with tile.TileContext(nc) as tc:
        with (
            tc.tile_pool(name="const", bufs=1) as cpool,
            tc.tile_pool(name="work", bufs=3) as wpool,
            tc.tile_pool(name="acc", bufs=1) as apool,
            tc.tile_pool(name="small", bufs=2) as opool,
            tc.tile_pool(name="psA", bufs=3, space="PSUM") as psA,
            tc.tile_pool(name="psS", bufs=4, space="PSUM") as psS,
            tc.tile_pool(name="psT", bufs=1, space="PSUM") as psT,
        ):
            # PE warm-up burst: dummy matmuls gated only by a DVE memset
            # (every engine exits the NEFF preamble ~7us; DVE has the fastest
            # memset), so the HAM clock gate starts ramping while the initial
            # DMA issues stream; real matmuls take over when operands land.
            scr = cpool.tile([128, SB], bf16)
            nc.vector.memset(scr[:], 0.0)
            ps_warm = psS.tile([128, SB], f32, tag="sims")
            for i in range(NWARM):
                nc.tensor.matmul(ps_warm[:], lhsT=scr[:, 0:128], rhs=scr[:],
                                 start=(i == 0), stop=(i == NWARM - 1))
            warm_sink = cpool.tile([128, 8], f32)
            nc.vector.tensor_copy(warm_sink[:], ps_warm[:, 0:8])

            # ---- consts. The first chunk-pair of cue + weights are the
            # first issues on the sync/scalar HWDGE queues so real matmuls
            # start as soon as the clock ramp and DMA latency allow. ----
            xt0 = wpool.tile([128, DCH, SB], f8, tag="xt")
            nc.sync.dma_start(xt0[:, 0:2, :], cueP[:, 0, 0:2, :])
            w1pt = cpool.tile([128, NPAIR, 2, MW], f8)
            nc.scalar.dma_start(w1pt[:, 0], w1P[:, 0])
            onesE = cpool.tile([128, 1], bf16)
            nc.vector.memset(onesE[:], 1.0)

            w2t = cpool.tile([128, 2, E], f8)
            iw1t = cpool.tile([128, 2 if has_ist else 1, 64], bf16)
            iw2t = cpool.tile([64, 1], bf16)
            ib1t = cpool.tile([64, 1], f32)
            centTt = cpool.tile([128, N], bf16)
            emot = cpool.tile([128, BL // 128, 4], f32)
            xtailT = cpool.tile([6, BL], bf16)
            # small/late-deadline consts ride the gpsimd SWDGE queue so the
            # scalar HWDGE queue stays short (its second ACT table load must
            # run before the first gelu at ~15us); earliest deadline first
            biast = cpool.tile([128, 4], f32)
            nc.gpsimd.dma_start(iw1t[:], iw1[:])
            nc.gpsimd.dma_start(xtailT[:], tailT[:])
            nc.gpsimd.dma_start(biast[:], bias[:])
            nc.gpsimd.dma_start(ib1t[:], ib1[:])
            nc.gpsimd.dma_start(iw2t[:], iw2[:])
            nc.gpsimd.dma_start(centTt[:], centT[:])
            nc.gpsimd.dma_start(emot[:], emo[:])

            # accumulators
            ssq_all = apool.tile([128, XT], f32)
            ic_all = apool.tile([128, XT], f32)
            esum_all = apool.tile([128, XT], f32)
            top8_all = apool.tile([128, XT, 8], f32)
            rinv_all = apool.tile([128, XT], f32)
            u_all = apool.tile([128, XT], f32)
            # const tiles for the GpSimd chains (tensor_tensor-only ALU)
            kmag = cpool.tile([128, 16], i32)
            nc.gpsimd.memset(kmag[:], 0x5F3759DF)
            chalf = cpool.tile([128, 8], f32)
            nc.gpsimd.memset(chalf[:], 0.5)
            c15 = cpool.tile([128, 8], f32)
            nc.gpsimd.memset(c15[:], 1.5)
            cone = cpool.tile([128, 8], f32)
            nc.gpsimd.memset(cone[:], 1.0)
            cm112 = cpool.tile([128, 8], f32)
            nc.gpsimd.memset(cm112[:], -1.0 / 12.0)
            c116 = cpool.tile([128, 8], f32)
            nc.gpsimd.memset(c116[:], 0.0625)
            c18 = cpool.tile([128, 8], f32)
            nc.gpsimd.memset(c18[:], 0.125)

            def seed_dve(ssq_sl, X, tagsfx):
                # quake seed on DVE int ops; returns the seed tile
                y0i = opool.tile([128, X], i32, tag="y0" + tagsfx)
                nc.vector.tensor_single_scalar(
                    y0i[:], ssq_sl.bitcast(i32), 1, AO.logical_shift_right)
                nc.vector.tensor_tensor(
                    y0i[:], kmag[:, 0:X], y0i[:], AO.subtract)
                return y0i

            def rsqrt_dve(ssq_sl, out_sl, X, tagsfx):
                # rinv = rsqrt(ssq): quake seed + 1 Newton step on DVE
                y0i = seed_dve(ssq_sl, X, tagsfx)
                hx = opool.tile([128, X], f32, tag="hx" + tagsfx)
                nc.vector.tensor_scalar_mul(hx[:], ssq_sl, 0.5)
                rs_t = opool.tile([128, X], f32, tag="rt" + tagsfx)
                cur = y0i[:].bitcast(f32)
                nc.vector.tensor_mul(rs_t[:], cur, cur)
                nc.vector.tensor_mul(rs_t[:], rs_t[:], hx[:])
                nc.vector.tensor_scalar(
                    rs_t[:], rs_t[:], -1.0, 1.5, AO.mult, AO.add)
                nc.vector.tensor_mul(out_sl, rs_t[:], cur)

            def sig_poly_dve(zsl, usl, X, tagsfx):
                # u = 0.125 + z/16 - z^3/192  (= sigmoid(z)/4, |z|<0.25)
                z2 = opool.tile([128, X], f32, tag="sz" + tagsfx)
                nc.vector.tensor_mul(z2[:], zsl, zsl)
                nc.vector.tensor_scalar(
                    z2[:], z2[:], -1.0 / 12.0, 1.0, AO.mult, AO.add)
                nc.vector.tensor_mul(usl, zsl, z2[:])
                nc.vector.tensor_scalar(
                    usl, usl, 0.0625, 0.125, AO.mult, AO.add)

            def rsqrt_gp(ssq_sl, out_sl, X, tagsfx):
                # seed on DVE (int shifts), 1 Newton step on the idle GpSimd
                y0i = seed_dve(ssq_sl, X, tagsfx)
                hx = opool.tile([128, X], f32, tag="hx" + tagsfx)
                nc.gpsimd.tensor_mul(hx[:], ssq_sl, chalf[:, 0:X])
                rs_t = opool.tile([128, X], f32, tag="rt" + tagsfx)
                cur = y0i[:].bitcast(f32)
                nc.gpsimd.tensor_mul(rs_t[:], cur, cur)
                nc.gpsimd.tensor_mul(rs_t[:], rs_t[:], hx[:])
                nc.gpsimd.tensor_tensor(
                    rs_t[:], c15[:, 0:X], rs_t[:], AO.subtract)
                nc.gpsimd.tensor_mul(out_sl, rs_t[:], cur)

            def sig_gp(zsl, usl, X, tagsfx):
                # same polynomial, tensor_tensor-only for GpSimd
                t = opool.tile([128, X], f32, tag="sg" + tagsfx)
                nc.gpsimd.tensor_mul(t[:], zsl, zsl)
                nc.gpsimd.tensor_mul(t[:], t[:], cm112[:, 0:X])
                nc.gpsimd.tensor_tensor(t[:], t[:], cone[:, 0:X], AO.add)
                nc.gpsimd.tensor_mul(t[:], t[:], zsl)
                nc.gpsimd.tensor_mul(t[:], t[:], c116[:, 0:X])
                nc.gpsimd.tensor_tensor(usl, t[:], c18[:, 0:X], AO.add)

            def assemble_ship(lo, hi, tagsfx, engine):
                # ot[:, :, 0:K] = top8 * rinv ; ot[:, :, K] = u * esum
                X = hi - lo
                ot = opool.tile([128, X, K + 1], f32, tag="ot" + tagsfx)
                eng = nc.gpsimd if engine == "gp" else nc.vector
                eng.tensor_mul(
                    ot[:, :, 0:K], top8_all[:, lo:hi, 0:K],
                    rinv_all[:, lo:hi].broadcast_to([128, X, K]))
                eng.tensor_mul(ot[:, :, K], u_all[:, lo:hi],
                               esum_all[:, lo:hi])
                # the DMA issue always rides the sync queue (idle once the
                # cue loads are done; a GpSimd SWDGE issue would block the
                # GpSimd compute stream for ~0.7us)
                nc.sync.dma_start(
                    out[:, lo * (K + 1) : hi * (K + 1)], ot[:])
                return ot

            for sb in range(NSB):
                if sb == 5:
                    # epilogue for tiles 0:16 in mid-run DVE slack
                    rsqrt_dve(ssq_all[:, 0:16], rinv_all[:, 0:16], 16, "a1")
                    sig_poly_dve(ic_all[:, 0:16], u_all[:, 0:16], 16, "a1")
                if sb == 6:
                    # ship tiles 0:16 (all deps landed by sb4); GpSimd
                    # assembles and issues the DMA so the sync queue stays
                    # clear for the remaining cue loads
                    assemble_ship(0, 16, "a1", "gp")
                    # epilogue for 16:24 (sb4-5 tiles) on the idle GpSimd
                    rsqrt_gp(ssq_all[:, 16:24], rinv_all[:, 16:24], 8, "b1")
                    sig_gp(ic_all[:, 16:24], u_all[:, 16:24], 8, "b1")
                if sb == 7:
                    # epilogue for 24:28 (sb6 tiles; ssq/z land mid-sb6):
                    # all on GpSimd, keeping DVE free for the max8 chain
                    rsqrt_gp(ssq_all[:, 24:28], rinv_all[:, 24:28], 4, "b2")
                    sig_gp(ic_all[:, 24:28], u_all[:, 24:28], 4, "b2")

                # ---- inputs. sb0 loads cue per k-tile-pair (fast ramp);
                # later sbs use one DMA. ----
                if sb == 0:
                    xt = xt0
                    nc.scalar.dma_start(w1pt[:, 1], w1P[:, 1])
                    nc.sync.dma_start(xt[:, 2:4, :], cueP[:, sb, 2:4, :])
                    nc.scalar.dma_start(w1pt[:, 2], w1P[:, 2])
                    nc.sync.dma_start(xt[:, 4:6, :], cueP[:, sb, 4:6, :])
                    nc.scalar.dma_start(w2t[:], w2[:])
                else:
                    xt = wpool.tile([128, DCH, SB], f8, tag="xt")
                    nc.sync.dma_start(xt[:], cueP[:, sb, :, :])
                xtail = xtailT[:, ts(sb, SB)]
                if has_ist:
                    xti = wpool.tile([128, SB], bf16, tag="xti")
                    nc.sync.dma_start(xti[:], istT[:, ts(sb, SB)])

                # ---- fused layer 1: [W1 | imp_w1_cue].T @ xT, fp8
                # DoubleRow; M-chunks 0,1 -> h1 halves, chunk 2 -> imp ----
                h1 = wpool.tile([128, 2, SB], f8, tag="h1")
                ps_imp = psA.tile([64, SB], f32, tag="mm")
                ps_h = [psA.tile([128, SB], f32, tag="mm", name=f"ps_h{i}")
                        for i in range(2)]
                for c in range(NPAIR):
                    pair = xt[:, 2 * c : 2 * c + 2, :]
                    nc.tensor.matmul(
                        ps_h[0][:], lhsT=w1pt[:, c, :, 0:128],
                        rhs=pair, start=(c == 0), stop=(c == NPAIR - 1),
                        perf_mode=DR,
                    )
                    nc.tensor.matmul(
                        ps_h[1][:], lhsT=w1pt[:, c, :, 128:256],
                        rhs=pair, start=(c == 0), stop=(c == NPAIR - 1),
                        perf_mode=DR,
                    )
                    nc.tensor.matmul(
                        ps_imp[:], lhsT=w1pt[:, c, :, 256:320],
                        rhs=pair, start=(c == 0), stop=False, perf_mode=DR,
                    )
                if has_ist:
                    nc.tensor.matmul(
                        ps_imp[:], lhsT=iw1t[:, 1, :], rhs=xti[:],
                        start=False, stop=False,
                    )
                nc.tensor.matmul(
                    ps_imp[:], lhsT=iw1t[0:6, 0, :], rhs=xtail,
                    start=False, stop=True,
                )
                for half in range(2):
                    nc.scalar.activation(
                        h1[:, half, :], ps_h[half][:], AF.Gelu,
                        bias=biast[:, half : half + 1], scale=SINV,
                    )
                himp = wpool.tile([64, SB], bf16, tag="himp")

                # ---- encoder layer 2: encT = W2.T @ h1T + b2, one fp8
                # DoubleRow matmul; descale + b2 + bf16 ride ACT ----
                ps_enc = psA.tile([128, SB], f32, tag="mm")
                nc.tensor.matmul(
                    ps_enc[:], lhsT=w2t[:, 0:2, :], rhs=h1[:, 0:2, :],
                    start=True, stop=True, perf_mode=DR,
                )
                # encb = psum/SC_W + b2 on DVE (tensor_scalar with per-
                # partition bias AP): ACT at ~82% busy is closer to the
                # critical path than DVE, and this unblocks L2->sims sooner
                encb = wpool.tile([128, SB], bf16, tag="encb")
                nc.vector.tensor_scalar(encb[:], ps_enc[:], 1.0 / SC_W,
                                        biast[:, 2:3], AO.mult, AO.add)
                nc.scalar.activation(himp[:], ps_imp[:], AF.Gelu,
                                     bias=ib1t[:], scale=SINV)
                enc2 = wpool.tile([128, SB], bf16, tag="enc2")
                nc.vector.tensor_mul(enc2[:], encb[:], encb[:])

                def ssq_block():
                    # ---- ||enc||^2 via PE ----
                    ps_ssq = psT.tile([128, Q], f32, tag="tiny")
                    for q in range(Q):
                        nc.tensor.matmul(
                            ps_ssq[:, q : q + 1],
                            lhsT=enc2[:, ts(q, 128)],
                            rhs=onesE[:],
                            start=True,
                            stop=True,
                        )
                    # PSUM->SBUF copy on ACT (Identity, bias 0) so the DVE
                    # stream stays pure max8s through the tail
                    nc.scalar.activation(ssq_all[:, ts(sb, Q)], ps_ssq[:],
                                         AF.Identity)

                # ---- importance head: z = himp @ iw2 + imp_b2 ----
                def imp_head():
                    ps_ic = psT.tile([128, Q], f32, tag="tiny")
                    for q in range(Q):
                        nc.tensor.matmul(
                            ps_ic[:, q : q + 1],
                            lhsT=himp[:, ts(q, 128)],
                            rhs=iw2t[:],
                            start=True,
                            stop=True,
                        )
                    nc.scalar.activation(ic_all[:, ts(sb, Q)], ps_ic[:],
                                         AF.Identity, bias=biast[:, 3:4])

                # ---- sims + top8 per 128-row tile; emitted before the
                # ssq/imp tiny matmuls so the PE produces the sims PSUMs
                # (which pace the DVE max8 chain) first ----
                for q in range(Q):
                    ps_sims = psS.tile([128, N], f32, tag="sims")
                    nc.tensor.matmul(
                        ps_sims[:],
                        lhsT=encb[:, ts(q, 128)],
                        rhs=centTt[:],
                        start=True,
                        stop=True,
                    )
                    nc.vector.max(top8_all[:, sb * Q + q, :], ps_sims[:])

                if sb < NSB - 1:
                    ssq_block()
                    imp_head()
                else:
                    X0 = (NSB - 1) * Q  # 28
                    ssq_block()
                    imp_head()
                    # rinv for 28:32: DVE seed right after the ssq copy,
                    # Newton on GpSimd in parallel with the max8 chain
                    rsqrt_gp(ssq_all[:, X0:XT], rinv_all[:, X0:XT], Q, "b3")

                    # final tile: sigmoid + both muls on DVE right after the
                    # last max8, then one small DMA. Emitted (and shipped)
                    # BEFORE the 16:28 ship so the sync queue issues the
                    # critical last DMA first - its completion semaphore
                    # gates the end-of-kernel barrier.
                    ot2 = opool.tile([128, Q, K + 1], f32, tag="ot_b")
                    sig_poly_dve(ic_all[:, X0:XT], u_all[:, X0:XT], Q, "b3")
                    nc.vector.tensor_mul(ot2[:, :, K], u_all[:, X0:XT],
                                         esum_all[:, X0:XT])
                    nc.vector.tensor_mul(
                        ot2[:, :, 0:K], top8_all[:, X0:XT, 0:K],
                        rinv_all[:, X0:XT].broadcast_to([128, Q, K]))
                    nc.sync.dma_start(out[:, X0 * (K + 1) :], ot2[:])

                    # ship 16:28 right after on DVE (free post-max8, while
                    # GpSimd is still on the 28:32 Newton); only needs to
                    # beat the end-of-kernel barrier
                    assemble_ship(16, 28, "a2", "dve")

                if sb == 0:
                    nc.vector.reduce_sum(
                        esum_all[:], emot[:], axis=mybir.AxisListType.X
                    )

    nc.compile()
    return nc


def _prep_inputs(has_ist, cue, internal_state, reward, timestamp,
                 emotional_state, centroids, enc_w1, enc_b1, enc_w2, enc_b2,
                 imp_w1, imp_b1, imp_w2, imp_b2):
    f32 = np.float32

    tail = np.empty((6, B), dtype=f32)
    tail[0] = reward[:, 0]
    tail[1] = timestamp[:, 0]
    tail[2:6] = emotional_state.T
    tail_bf = tail.astype(BF16)
    cue_q = np.clip(cue * SC_X, -240.0, 240.0).astype(FP8)
    ist_bf = internal_state.astype(BF16) if has_ist else None

    w1e = np.concatenate([enc_w1, imp_w1[:D]], axis=1)       # [768, 320]
    w1 = np.ascontiguousarray(
        np.clip(w1e * SC_W, -240.0, 240.0).astype(FP8)
        .reshape(DCH, 128, MW).transpose(1, 0, 2)
    )                                                        # [128, DCH, 320]
    w1Pm = np.ascontiguousarray(w1.reshape(128, NPAIR, 2, MW))
    w2 = np.ascontiguousarray(
        np.clip(enc_w2 * SC_W, -240.0, 240.0).astype(FP8)
        .reshape(2, 128, E).transpose(1, 0, 2)
    )
    # imp tail / istate chunks stay bf16 but share the fp8-scaled PSUM:
    # pre-scale their weights by SC_X*SC_W so Gelu(psum*SINV+b) is exact.
    S = SC_X * SC_W
    nchi = 2 if has_ist else 1
    iw1p = np.zeros((nchi * 128, 64), dtype=f32)
    iw1p[0:6] = imp_w1[TOT - 6 : TOT] * S        # chunk 0 = reward/ts/emo tail
    if has_ist:
        iw1p[128 : 128 + E] = imp_w1[D : D + E] * S  # chunk 1 = internal_state
    iw1 = np.ascontiguousarray(
        iw1p.astype(BF16).reshape(nchi, 128, 64).transpose(1, 0, 2)
    )
    iw2 = np.ascontiguousarray(imp_w2.astype(BF16).reshape(64, 1))
    bias = np.empty((128, 4), dtype=f32)
    bias[:, 0:2] = enc_b1.astype(f32).reshape(2, 128).T
    bias[:, 2] = enc_b2.astype(f32)
    bias[:, 3] = float(np.asarray(imp_b2).reshape(-1)[0])
    ib1 = np.ascontiguousarray(imp_b1.astype(f32).reshape(64, 1))

    cn = np.linalg.norm(centroids.astype(f32), axis=1)
    centT = np.ascontiguousarray((centroids / cn[:, None]).T).astype(BF16)

    shared = dict(w1P=w1Pm, w2=w2, iw1=iw1, iw2=iw2, bias=bias,
                  ib1=ib1, centT=centT)
    in_maps = []
    for i in range(N_CORES):
        sl = slice(i * BL, (i + 1) * BL)
        m = dict(shared)
        # cueP[p, sb, c, b] = cue[sb*SB+b, c*128+p] (per-sb contiguous)
        m["cueP"] = np.ascontiguousarray(
            cue_q[sl].T.reshape(DCH, 128, NSB, SB).transpose(1, 2, 0, 3)
        )
        m["tailT"] = np.ascontiguousarray(tail_bf[:, sl])
        if has_ist:
            m["istT"] = np.ascontiguousarray(ist_bf[sl].T)
        # device-friendly emo layout: emo_dev[p, x, e] = emotional[x*128+p, e]
        m["emo"] = np.ascontiguousarray(
            emotional_state[sl].astype(f32).reshape(BL // 128, 128, 4)
            .transpose(1, 0, 2)
        )
        in_maps.append(m)
    return in_maps


def kernel(cue, internal_state, reward, timestamp, emotional_state, centroids,
           enc_w1, enc_b1, enc_w2, enc_b2, imp_w1, imp_b1, imp_w2, imp_b2,
           top_k, **run_kwargs):
    assert int(top_k) == K, f"kernel hardcodes top_k={K}, got {top_k}"
    from concourse.bass_utils import run_bass_kernel_spmd

    has_ist = bool(np.any(internal_state))
    if ("nc", has_ist) not in _CACHE:
        _CACHE[("nc", has_ist)] = _build_nc(has_ist)
    nc = _CACHE[("nc", has_ist)]

    in_maps = _prep_inputs(
        has_ist,
        np.asarray(cue, np.float32), np.asarray(internal_state, np.float32),
        np.asarray(reward, np.float32), np.asarray(timestamp, np.float32),
        np.asarray(emotional_state, np.float32),
        np.asarray(centroids, np.float32),
        np.asarray(enc_w1, np.float32), np.asarray(enc_b1, np.float32),
        np.asarray(enc_w2, np.float32), np.asarray(enc_b2, np.float32),
        np.asarray(imp_w1, np.float32), np.asarray(imp_b1, np.float32),
        np.asarray(imp_w2, np.float32), np.asarray(imp_b2, np.float32),
    )
    res = run_bass_kernel_spmd(
        nc, in_maps, core_ids=list(range(N_CORES)), **run_kwargs
    )
    # device out is [128, XT*6] with out_dev[p, x*6+j] = out[x*128+p, j]
    parts = []
    for i in range(N_CORES):
        od = res.results[i]["out"].reshape(128, BL // 128, K + 1)
        parts.append(np.ascontiguousarray(od.transpose(1, 0, 2)).reshape(BL, K + 1))
    out = np.concatenate(parts, axis=0)
    _CACHE["last_results"] = res
    return out


# revision 28
# speedup vs baseline: 1.1221x; 1.1221x over previous
"""Trainium2 Bass kernel for the DifferentiableMemory scatter_memory problem.

Data-parallel over 8 NeuronCores: batch B=32768 is sharded into 8 x 4096 rows.
Host side does layout only (transpose/cast/concat/weight repack); all NN math
(encoder MLP, cosine sims, top-k, importance net) runs on device with fp32
PSUM accumulation.

Device dataflow (per core, 8 superblocks of 512 batch columns):
  activations live transposed [feature, batch]:
    xT        [128, 6, 512]  cue.T chunks in fp8 e4m3 (x32 host prescale),
                DRAM laid out per-superblock-contiguous
    h1T       = gelu((W1.T @ xT)/S + b1)    -> [256, 512] bf16, W1 fp8 x512,
                3 DoubleRow matmuls per output tile, weights packed pair-major
                so the first chunk-pair's 48KB lands early and matmuls start
                ~2us sooner than a monolithic weight load
    encT      = W2.T @ h1T + b2             -> [128, 512] bf16 (one fp8 DR
                matmul; descale+bias ride the ACT Identity)
    enc2      = encT^2 via ACT Square (same gelu_and_others table: no swaps);
                ssq[b] = ones.T @ enc2 on PE; rsqrt = quake seed + 1 Newton
                step (rel err ~2e-3, far under the output budget)
    sims[b,n] = encT.T @ centT_scaled -> [128, 500] fp32 PSUM (centT
                pre-divided by ||c||; divide by ||enc|| AFTER top-8: positive
                per-row scale preserves order), top8 = one DVE max8 per tile
    impT      = sigmoid(w2i.T @ gelu((W1i.T @ xT)/S + b1i) + b2i) * mean(emo)
                sigmoid as polynomial 0.5+z/4-z^3/48 (|z|<0.25, err <1e-5);
                the /4 of mean(emo) is folded into the polynomial

  Epilogue is pipelined off the critical path: rsqrt/sigmoid chains run in
  DVE slack (tiles 0:16 at sb5) or on the idle GpSimd (16:24 at sb6, 24:32
  at sb7), and the output ships in three staged DMAs (tiles 0:16 during sb6,
  16:28 during sb7's matmuls, 28:32 right after the last max8) so the
  tail after the final matmul is just max8 -> one mul -> one small DMA.
  A PE warm-up burst (gated by a GpSimd memset, ~1.1us earlier than a DVE
  one) rides out the HAM clock ramp before the first real matmuls.
"""

import numpy as np
import ml_dtypes

BF16 = ml_dtypes.bfloat16
FP8 = ml_dtypes.float8_e4m3

N_CORES = 8
B = 32768
BL = B // N_CORES          # 4096 rows per core
SB = 512                   # superblock: batch columns per iteration
NSB = BL // SB             # 8 superblocks
Q = SB // 128              # 4 x 128-row tiles per superblock
D = 768
H1 = 256
E = 128
N = 500
K = 5
TOT = 902
DCH = D // 128             # 6
NPAIR = DCH // 2           # 3 chunk-pairs for DoubleRow
MW = H1 + 64               # 320 fused layer-1 output cols (h0|h1|imp)
SC_X = 32.0                # fp8 prescale on cue
SC_W = 512.0               # fp8 prescale on layer-1 weights
SINV = 1.0 / (SC_X * SC_W)
NWARM = 8

_CACHE = {}


def _build_nc(has_ist):
    """Build the device kernel. has_ist: include the internal_state chunk
    (False when it is all-zeros, making its contribution exactly zero)."""
    import concourse.bacc as bacc
    import concourse.bass as bass
    import concourse.tile as tile
    from concourse import mybir

    f32 = mybir.dt.float32
    bf16 = mybir.dt.bfloat16
    f8 = mybir.dt.float8e4
    i32 = mybir.dt.int32
    AF = mybir.ActivationFunctionType
    AO = mybir.AluOpType
    DR = mybir.MatmulPerfMode.DoubleRow
    ts = bass.ts

    nc = bacc.Bacc(None, target_bir_lowering=False,
               enable_asserts=False, enable_partition_id=False)

    # cue pre-chunked on host: cueP[p, sb, c, b] = cue[sb*SB+b, c*128+p]
    cueP = nc.dram_tensor("cueP", [128, NSB, DCH, SB], f8, kind="ExternalInput")
    tailT = nc.dram_tensor("tailT", [6, BL], bf16, kind="ExternalInput")
    if has_ist:
        istT = nc.dram_tensor("istT", [E, BL], bf16, kind="ExternalInput")
    emo = nc.dram_tensor("emo", [128, BL // 128, 4], f32, kind="ExternalInput")
    # fused layer-1 weights, chunk-PAIR major so each DR pass's weights are
    # one small contiguous DMA: w1P[p, c, j, m] = w1[p, 2c+j, m]
    w1P = nc.dram_tensor("w1P", [128, NPAIR, 2, MW], f8, kind="ExternalInput")
    w2 = nc.dram_tensor("w2", [128, 2, E], f8, kind="ExternalInput")
    iw1 = nc.dram_tensor("iw1", [128, 2 if has_ist else 1, 64], bf16,
                         kind="ExternalInput")
    iw2 = nc.dram_tensor("iw2", [64, 1], bf16, kind="ExternalInput")
    # bias_all: col 0-1 = enc_b1 halves, col 2 = enc_b2, col 3 = imp_b2
    bias = nc.dram_tensor("bias", [128, 4], f32, kind="ExternalInput")
    ib1 = nc.dram_tensor("ib1", [64, 1], f32, kind="ExternalInput")
    centT = nc.dram_tensor("centT", [128, N], bf16, kind="ExternalInput")
    out = nc.dram_tensor("out", [128, (BL // 128) * (K + 1)], f32,
                         kind="ExternalOutput")

    XT = NSB * Q  # 32 tiles of 128 rows

    with tile.TileContext(nc) as tc:
        with (
            tc.tile_pool(name="const", bufs=1) as cpool,
            tc.tile_pool(name="work", bufs=3) as wpool,
            tc.tile_pool(name="acc", bufs=1) as apool,
            tc.tile_pool(name="small", bufs=2) as opool,
            tc.tile_pool(name="psA", bufs=3, space="PSUM") as psA,
            tc.tile_pool(name="psS", bufs=4, space="PSUM") as psS,
            tc.tile_pool(name="psT", bufs=1, space="PSUM") as psT,
        ):
            # PE warm-up burst: dummy matmuls gated only by a DVE memset
            # (every engine exits the NEFF preamble ~7us; DVE has the fastest
            # memset), so the HAM clock gate starts ramping while the initial
            # DMA issues stream; real matmuls take over when operands land.
            scr = cpool.tile([128, SB], bf16)
            nc.vector.memset(scr[:], 0.0)
            ps_warm = psS.tile([128, SB], f32, tag="sims")
            for i in range(NWARM):
                nc.tensor.matmul(ps_warm[:], lhsT=scr[:, 0:128], rhs=scr[:],
                                 start=(i == 0), stop=(i == NWARM - 1))
            warm_sink = cpool.tile([128, 8], f32)
            nc.vector.tensor_copy(warm_sink[:], ps_warm[:, 0:8])

            # ---- consts. The first chunk-pair of cue + weights are the
            # first issues on the sync/scalar HWDGE queues so real matmuls
            # start as soon as the clock ramp and DMA latency allow. ----
            xt0 = wpool.tile([128, DCH, SB], f8, tag="xt")
            nc.sync.dma_start(xt0[:, 0:2, :], cueP[:, 0, 0:2, :])
            w1pt = cpool.tile([128, NPAIR, 2, MW], f8)
            nc.scalar.dma_start(w1pt[:, 0], w1P[:, 0])
            onesE = cpool.tile([128, 1], bf16)
            nc.vector.memset(onesE[:], 1.0)

            w2t = cpool.tile([128, 2, E], f8)
            iw1t = cpool.tile([128, 2 if has_ist else 1, 64], bf16)
            iw2t = cpool.tile([64, 1], bf16)
            ib1t = cpool.tile([64, 1], f32)
            centTt = cpool.tile([128, N], bf16)
            emot = cpool.tile([128, BL // 128, 4], f32)
            xtailT = cpool.tile([6, BL], bf16)
            # small/late-deadline consts ride the gpsimd SWDGE queue so the
            # scalar HWDGE queue stays short (its second ACT table load must
            # run before the first gelu at ~15us); earliest deadline first
            biast = cpool.tile([128, 4], f32)
            nc.gpsimd.dma_start(iw1t[:], iw1[:])
            nc.gpsimd.dma_start(xtailT[:], tailT[:])
            nc.gpsimd.dma_start(biast[:], bias[:])
            nc.gpsimd.dma_start(ib1t[:], ib1[:])
            nc.gpsimd.dma_start(iw2t[:], iw2[:])
            nc.gpsimd.dma_start(centTt[:], centT[:])
            nc.gpsimd.dma_start(emot[:], emo[:])

            # accumulators
            ssq_all = apool.tile([128, XT], f32)
            ic_all = apool.tile([128, XT], f32)
            esum_all = apool.tile([128, XT], f32)
            top8_all = apool.tile([128, XT, 8], f32)
            rinv_all = apool.tile([128, XT], f32)
            u_all = apool.tile([128, XT], f32)
            # const tiles for the GpSimd chains (tensor_tensor-only ALU)
            kmag = cpool.tile([128, 16], i32)
            nc.gpsimd.memset(kmag[:], 0x5F3759DF)
            chalf = cpool.tile([128, 8], f32)
            nc.gpsimd.memset(chalf[:], 0.5)
            c15 = cpool.tile([128, 8], f32)
            nc.gpsimd.memset(c15[:], 1.5)
            cone = cpool.tile([128, 8], f32)
            nc.gpsimd.memset(cone[:], 1.0)
            cm112 = cpool.tile([128, 8], f32)
            nc.gpsimd.memset(cm112[:], -1.0 / 12.0)
            c116 = cpool.tile([128, 8], f32)
            nc.gpsimd.memset(c116[:], 0.0625)
            c18 = cpool.tile([128, 8], f32)
            nc.gpsimd.memset(c18[:], 0.125)

            def seed_dve(ssq_sl, X, tagsfx):
                # quake seed on DVE int ops; returns the seed tile
                y0i = opool.tile([128, X], i32, tag="y0" + tagsfx)
                nc.vector.tensor_single_scalar(
                    y0i[:], ssq_sl.bitcast(i32), 1, AO.logical_shift_right)
                nc.vector.tensor_tensor(
                    y0i[:], kmag[:, 0:X], y0i[:], AO.subtract)
                return y0i

            def rsqrt_dve(ssq_sl, out_sl, X, tagsfx):
                # rinv = rsqrt(ssq): quake seed + 1 Newton step on DVE
                y0i = seed_dve(ssq_sl, X, tagsfx)
                hx = opool.tile([128, X], f32, tag="hx" + tagsfx)
                nc.vector.tensor_scalar_mul(hx[:], ssq_sl, 0.5)
                rs_t = opool.tile([128, X], f32, tag="rt" + tagsfx)
                cur = y0i[:].bitcast(f32)
                nc.vector.tensor_mul(rs_t[:], cur, cur)
                nc.vector.tensor_mul(rs_t[:], rs_t[:], hx[:])
                nc.vector.tensor_scalar(
                    rs_t[:], rs_t[:], -1.0, 1.5, AO.mult, AO.add)
                nc.vector.tensor_mul(out_sl, rs_t[:], cur)

            def sig_poly_dve(zsl, usl, X, tagsfx):
                # u = 0.125 + z/16 - z^3/192  (= sigmoid(z)/4, |z|<0.25)
                z2 = opool.tile([128, X], f32, tag="sz" + tagsfx)
                nc.vector.tensor_mul(z2[:], zsl, zsl)
                nc.vector.tensor_scalar(
                    z2[:], z2[:], -1.0 / 12.0, 1.0, AO.mult, AO.add)
                nc.vector.tensor_mul(usl, zsl, z2[:])
                nc.vector.tensor_scalar(
                    usl, usl, 0.0625, 0.125, AO.mult, AO.add)

            def rsqrt_gp(ssq_sl, out_sl, X, tagsfx):
                # seed on DVE (int shifts), 1 Newton step on the idle GpSimd
                y0i = seed_dve(ssq_sl, X, tagsfx)
                hx = opool.tile([128, X], f32, tag="hx" + tagsfx)
                nc.gpsimd.tensor_mul(hx[:], ssq_sl, chalf[:, 0:X])
                rs_t = opool.tile([128, X], f32, tag="rt" + tagsfx)
                cur = y0i[:].bitcast(f32)
                nc.gpsimd.tensor_mul(rs_t[:], cur, cur)
                nc.gpsimd.tensor_mul(rs_t[:], rs_t[:], hx[:])
                nc.gpsimd.tensor_tensor(
                    rs_t[:], c15[:, 0:X], rs_t[:], AO.subtract)
                nc.gpsimd.tensor_mul(out_sl, rs_t[:], cur)

            def sig_gp(zsl, usl, X, tagsfx):
                # same polynomial, tensor_tensor-only for GpSimd
                t = opool.tile([128, X], f32, tag="sg" + tagsfx)
                nc.gpsimd.tensor_mul(t[:], zsl, zsl)
                nc.gpsimd.tensor_mul(t[:], t[:], cm112[:, 0:X])
                nc.gpsimd.tensor_tensor(t[:], t[:], cone[:, 0:X], AO.add)
                nc.gpsimd.tensor_mul(t[:], t[:], zsl)
                nc.gpsimd.tensor_mul(t[:], t[:], c116[:, 0:X])
                nc.gpsimd.tensor_tensor(usl, t[:], c18[:, 0:X], AO.add)

            def assemble_ship(lo, hi, tagsfx, engine):
                # ot[:, :, 0:K] = top8 * rinv ; ot[:, :, K] = u * esum
                X = hi - lo
                ot = opool.tile([128, X, K + 1], f32, tag="ot" + tagsfx)
                eng = nc.gpsimd if engine == "gp" else nc.vector
                eng.tensor_mul(
                    ot[:, :, 0:K], top8_all[:, lo:hi, 0:K],
                    rinv_all[:, lo:hi].broadcast_to([128, X, K]))
                eng.tensor_mul(ot[:, :, K], u_all[:, lo:hi],
                               esum_all[:, lo:hi])
                # the DMA issue always rides the sync queue (idle once the
                # cue loads are done; a GpSimd SWDGE issue would block the
                # GpSimd compute stream for ~0.7us)
                nc.sync.dma_start(
                    out[:, lo * (K + 1) : hi * (K + 1)], ot[:])
                return ot

            for sb in range(NSB):
                if sb == 5:
                    # epilogue for tiles 0:16 in mid-run DVE slack
                    rsqrt_dve(ssq_all[:, 0:16], rinv_all[:, 0:16], 16, "a1")
                    sig_poly_dve(ic_all[:, 0:16], u_all[:, 0:16], 16, "a1")
                if sb == 6:
                    # ship tiles 0:16 (all deps landed by sb4); GpSimd
                    # assembles and issues the DMA so the sync queue stays
                    # clear for the remaining cue loads
                    assemble_ship(0, 16, "a1", "gp")
                    # epilogue for 16:24 (sb4-5 tiles) on the idle GpSimd
                    rsqrt_gp(ssq_all[:, 16:24], rinv_all[:, 16:24], 8, "b1")
                    sig_gp(ic_all[:, 16:24], u_all[:, 16:24], 8, "b1")
                if sb == 7:
                    # epilogue for 24:28 (sb6 tiles; ssq/z land mid-sb6):
                    # all on GpSimd, keeping DVE free for the max8 chain
                    rsqrt_gp(ssq_all[:, 24:28], rinv_all[:, 24:28], 4, "b2")
                    sig_gp(ic_all[:, 24:28], u_all[:, 24:28], 4, "b2")

                # ---- inputs. sb0 loads cue per k-tile-pair (fast ramp);
                # later sbs use one DMA. ----
                if sb == 0:
                    xt = xt0
                    nc.scalar.dma_start(w1pt[:, 1], w1P[:, 1])
                    nc.sync.dma_start(xt[:, 2:4, :], cueP[:, sb, 2:4, :])
                    nc.scalar.dma_start(w1pt[:, 2], w1P[:, 2])
                    nc.sync.dma_start(xt[:, 4:6, :], cueP[:, sb, 4:6, :])
                    nc.scalar.dma_start(w2t[:], w2[:])
                else:
                    xt = wpool.tile([128, DCH, SB], f8, tag="xt")
                    nc.sync.dma_start(xt[:], cueP[:, sb, :, :])
                xtail = xtailT[:, ts(sb, SB)]
                if has_ist:
                    xti = wpool.tile([128, SB], bf16, tag="xti")
                    nc.sync.dma_start(xti[:], istT[:, ts(sb, SB)])

                # ---- fused layer 1: [W1 | imp_w1_cue].T @ xT, fp8
                # DoubleRow; M-chunks 0,1 -> h1 halves, chunk 2 -> imp ----
                h1 = wpool.tile([128, 2, SB], f8, tag="h1")
                ps_imp = psA.tile([64, SB], f32, tag="mm")
                ps_h = [psA.tile([128, SB], f32, tag="mm", name=f"ps_h{i}")
                        for i in range(2)]
                for c in range(NPAIR):
                    pair = xt[:, 2 * c : 2 * c + 2, :]
                    nc.tensor.matmul(
                        ps_h[0][:], lhsT=w1pt[:, c, :, 0:128],
                        rhs=pair, start=(c == 0), stop=(c == NPAIR - 1),
                        perf_mode=DR,
                    )
                    nc.tensor.matmul(
                        ps_h[1][:], lhsT=w1pt[:, c, :, 128:256],
                        rhs=pair, start=(c == 0), stop=(c == NPAIR - 1),
                        perf_mode=DR,
                    )
                    nc.tensor.matmul(
                        ps_imp[:], lhsT=w1pt[:, c, :, 256:320],
                        rhs=pair, start=(c == 0), stop=False, perf_mode=DR,
                    )
                if has_ist:
                    nc.tensor.matmul(
                        ps_imp[:], lhsT=iw1t[:, 1, :], rhs=xti[:],
                        start=False, stop=False,
                    )
                nc.tensor.matmul(
                    ps_imp[:], lhsT=iw1t[0:6, 0, :], rhs=xtail,
                    start=False, stop=True,
                )
                for half in range(2):
                    nc.scalar.activation(
                        h1[:, half, :], ps_h[half][:], AF.Gelu,
                        bias=biast[:, half : half + 1], scale=SINV,
                    )
                himp = wpool.tile([64, SB], bf16, tag="himp")

                # ---- encoder layer 2: encT = W2.T @ h1T + b2, one fp8
                # DoubleRow matmul; descale + b2 + bf16 ride ACT ----
                ps_enc = psA.tile([128, SB], f32, tag="mm")
                nc.tensor.matmul(
                    ps_enc[:], lhsT=w2t[:, 0:2, :], rhs=h1[:, 0:2, :],
                    start=True, stop=True, perf_mode=DR,
                )
                encb = wpool.tile([128, SB], bf16, tag="encb")
                nc.scalar.activation(encb[:], ps_enc[:], AF.Identity,
                                     bias=biast[:, 2:3], scale=1.0 / SC_W)
                nc.scalar.activation(himp[:], ps_imp[:], AF.Gelu,
                                     bias=ib1t[:], scale=SINV)
                enc2 = wpool.tile([128, SB], bf16, tag="enc2")
                nc.vector.tensor_mul(enc2[:], encb[:], encb[:])

                def ssq_block():
                    # ---- ||enc||^2 via PE ----
                    ps_ssq = psT.tile([128, Q], f32, tag="tiny")
                    for q in range(Q):
                        nc.tensor.matmul(
                            ps_ssq[:, q : q + 1],
                            lhsT=enc2[:, ts(q, 128)],
                            rhs=onesE[:],
                            start=True,
                            stop=True,
                        )
                    # PSUM->SBUF copy on ACT (Identity, bias 0) so the DVE
                    # stream stays pure max8s through the tail
                    nc.scalar.activation(ssq_all[:, ts(sb, Q)], ps_ssq[:],
                                         AF.Identity)

                # ---- importance head: z = himp @ iw2 + imp_b2 ----
                def imp_head():
                    ps_ic = psT.tile([128, Q], f32, tag="tiny")
                    for q in range(Q):
                        nc.tensor.matmul(
                            ps_ic[:, q : q + 1],
                            lhsT=himp[:, ts(q, 128)],
                            rhs=iw2t[:],
                            start=True,
                            stop=True,
                        )
                    nc.scalar.activation(ic_all[:, ts(sb, Q)], ps_ic[:],
                                         AF.Identity, bias=biast[:, 3:4])

                # ---- sims + top8 per 128-row tile; emitted before the
                # ssq/imp tiny matmuls so the PE produces the sims PSUMs
                # (which pace the DVE max8 chain) first ----
                for q in range(Q):
                    ps_sims = psS.tile([128, N], f32, tag="sims")
                    nc.tensor.matmul(
                        ps_sims[:],
                        lhsT=encb[:, ts(q, 128)],
                        rhs=centTt[:],
                        start=True,
                        stop=True,
                    )
                    nc.vector.max(top8_all[:, sb * Q + q, :], ps_sims[:])

                if sb < NSB - 1:
                    ssq_block()
                    imp_head()
                else:
                    X0 = (NSB - 1) * Q  # 28
                    ssq_block()
                    imp_head()
                    # rinv for 28:32: DVE seed right after the ssq copy,
                    # Newton on GpSimd in parallel with the max8 chain
                    rsqrt_gp(ssq_all[:, X0:XT], rinv_all[:, X0:XT], Q, "b3")

                    # final tile: sigmoid + both muls on DVE right after the
                    # last max8, then one small DMA. Emitted (and shipped)
                    # BEFORE the 16:28 ship so the sync queue issues the
                    # critical last DMA first - its completion semaphore
                    # gates the end-of-kernel barrier.
                    ot2 = opool.tile([128, Q, K + 1], f32, tag="ot_b")
                    sig_poly_dve(ic_all[:, X0:XT], u_all[:, X0:XT], Q, "b3")
                    nc.vector.tensor_mul(ot2[:, :, K], u_all[:, X0:XT],
                                         esum_all[:, X0:XT])
                    nc.vector.tensor_mul(
                        ot2[:, :, 0:K], top8_all[:, X0:XT, 0:K],
                        rinv_all[:, X0:XT].broadcast_to([128, Q, K]))
                    nc.sync.dma_start(out[:, X0 * (K + 1) :], ot2[:])

                    # ship 16:28 right after on DVE (free post-max8, while
                    # GpSimd is still on the 28:32 Newton); only needs to
                    # beat the end-of-kernel barrier
                    assemble_ship(16, 28, "a2", "dve")

                if sb == 0:
                    nc.vector.reduce_sum(
                        esum_all[:], emot[:], axis=mybir.AxisListType.X
                    )

    nc.compile()
    return nc


def _prep_inputs(has_ist, cue, internal_state, reward, timestamp,
                 emotional_state, centroids, enc_w1, enc_b1, enc_w2, enc_b2,
                 imp_w1, imp_b1, imp_w2, imp_b2):
    f32 = np.float32

    tail = np.empty((6, B), dtype=f32)
    tail[0] = reward[:, 0]
    tail[1] = timestamp[:, 0]
    tail[2:6] = emotional_state.T
    tail_bf = tail.astype(BF16)
    cue_q = np.clip(cue * SC_X, -240.0, 240.0).astype(FP8)
    ist_bf = internal_state.astype(BF16) if has_ist else None

    w1e = np.concatenate([enc_w1, imp_w1[:D]], axis=1)       # [768, 320]
    w1 = np.ascontiguousarray(
        np.clip(w1e * SC_W, -240.0, 240.0).astype(FP8)
        .reshape(DCH, 128, MW).transpose(1, 0, 2)
    )                                                        # [128, DCH, 320]
    w1Pm = np.ascontiguousarray(w1.reshape(128, NPAIR, 2, MW))
    w2 = np.ascontiguousarray(
        np.clip(enc_w2 * SC_W, -240.0, 240.0).astype(FP8)
        .reshape(2, 128, E).transpose(1, 0, 2)
    )
    # imp tail / istate chunks stay bf16 but share the fp8-scaled PSUM:
    # pre-scale their weights by SC_X*SC_W so Gelu(psum*SINV+b) is exact.
    S = SC_X * SC_W
    nchi = 2 if has_ist else 1
    iw1p = np.zeros((nchi * 128, 64), dtype=f32)
    iw1p[0:6] = imp_w1[TOT - 6 : TOT] * S        # chunk 0 = reward/ts/emo tail
    if has_ist:
        iw1p[128 : 128 + E] = imp_w1[D : D + E] * S  # chunk 1 = internal_state
    iw1 = np.ascontiguousarray(
        iw1p.astype(BF16).reshape(nchi, 128, 64).transpose(1, 0, 2)
    )
    iw2 = np.ascontiguousarray(imp_w2.astype(BF16).reshape(64, 1))
    bias = np.empty((128, 4), dtype=f32)
    bias[:, 0:2] = enc_b1.astype(f32).reshape(2, 128).T
    bias[:, 2] = enc_b2.astype(f32)
    bias[:, 3] = float(np.asarray(imp_b2).reshape(-1)[0])
    ib1 = np.ascontiguousarray(imp_b1.astype(f32).reshape(64, 1))

    cn = np.linalg.norm(centroids.astype(f32), axis=1)
    centT = np.ascontiguousarray((centroids / cn[:, None]).T).astype(BF16)

    shared = dict(w1P=w1Pm, w2=w2, iw1=iw1, iw2=iw2, bias=bias,
                  ib1=ib1, centT=centT)
    in_maps = []
    for i in range(N_CORES):
        sl = slice(i * BL, (i + 1) * BL)
        m = dict(shared)
        # cueP[p, sb, c, b] = cue[sb*SB+b, c*128+p] (per-sb contiguous)
        m["cueP"] = np.ascontiguousarray(
            cue_q[sl].T.reshape(DCH, 128, NSB, SB).transpose(1, 2, 0, 3)
        )
        m["tailT"] = np.ascontiguousarray(tail_bf[:, sl])
        if has_ist:
            m["istT"] = np.ascontiguousarray(ist_bf[sl].T)
        # device-friendly emo layout: emo_dev[p, x, e] = emotional[x*128+p, e]
        m["emo"] = np.ascontiguousarray(
            emotional_state[sl].astype(f32).reshape(BL // 128, 128, 4)
            .transpose(1, 0, 2)
        )
        in_maps.append(m)
    return in_maps


def kernel(cue, internal_state, reward, timestamp, emotional_state, centroids,
           enc_w1, enc_b1, enc_w2, enc_b2, imp_w1, imp_b1, imp_w2, imp_b2,
           top_k, **run_kwargs):
    assert int(top_k) == K, f"kernel hardcodes top_k={K}, got {top_k}"
    from concourse.bass_utils import run_bass_kernel_spmd

    has_ist = bool(np.any(internal_state))
    if ("nc", has_ist) not in _CACHE:
        _CACHE[("nc", has_ist)] = _build_nc(has_ist)
    nc = _CACHE[("nc", has_ist)]

    in_maps = _prep_inputs(
        has_ist,
        np.asarray(cue, np.float32), np.asarray(internal_state, np.float32),
        np.asarray(reward, np.float32), np.asarray(timestamp, np.float32),
        np.asarray(emotional_state, np.float32),
        np.asarray(centroids, np.float32),
        np.asarray(enc_w1, np.float32), np.asarray(enc_b1, np.float32),
        np.asarray(enc_w2, np.float32), np.asarray(enc_b2, np.float32),
        np.asarray(imp_w1, np.float32), np.asarray(imp_b1, np.float32),
        np.asarray(imp_w2, np.float32), np.asarray(imp_b2, np.float32),
    )
    res = run_bass_kernel_spmd(
        nc, in_maps, core_ids=list(range(N_CORES)), **run_kwargs
    )
    # device out is [128, XT*6] with out_dev[p, x*6+j] = out[x*128+p, j]
    parts = []
    for i in range(N_CORES):
        od = res.results[i]["out"].reshape(128, BL // 128, K + 1)
        parts.append(np.ascontiguousarray(od.transpose(1, 0, 2)).reshape(BL, K + 1))
    out = np.concatenate(parts, axis=0)
    _CACHE["last_results"] = res
    return out
